# revision 5
# baseline (speedup 1.0000x reference)
"""Weighted BCE2D loss kernel for Trainium2 (8 NeuronCores, data-parallel).

Computes, for input p and binary target t of shape (32, 1, 1024, 1024) f32:

    pos = sum(t);  neg = S - pos;  S = p.size
    A = sum_{t=1} ln(p);  B = sum_{t=0} ln(1-p)
    loss = -(neg*A + pos*B) / S**2

which equals the reference
    -mean(w * (t*log(p) + (1-t)*log1p(-p))),  w = where(pos, neg/S, pos/S)
(the -100 log-clamp never fires: p is in [1e-4, 1-1e-4] so log >= -9.3).

Single pass over the data per core. Per element:
    u = p + t                (DVE tensor_tensor)
    u = |1 - u| = |p+t-1|    (ACT Abs, scale=-1 bias=1)  -> equals p if t=1 else 1-p
    l = ln(u)                (ACT Ln, bf16 out, fused f32 accum -> S1 partials)
    m = t * l                (DVE tensor_tensor, all-bf16 -> 2x mode)
    sum(m), sum(t) via PE bf16 matmuls with a ones vector (PSUM accumulate).
target is DMA-loaded as bf16 (SWDGE casts in flight; t in {0,1} is exact),
so the reduction matmuls are single-pass bf16 instead of split fp32.
Host combines the 8 cores' (S1, S2, S3) partials: A = S2, B = S1 - S2, pos = S3.
"""

import sys
import numpy as np

for _p in ("/opt/trn_rl_repo", "/root/.axon_site/_ro/trn_rl_repo"):
    if _p not in sys.path:
        sys.path.append(_p)

N_CORES = 8
N, C, H, W = 32, 1, 1024, 1024
S_TOTAL = N * C * H * W                 # 33_554_432
PER_CORE = S_TOTAL // N_CORES           # 4_194_304
F = 2048                                # tile free dim
P = 128                                 # partitions
NT = PER_CORE // (P * F)                # 16 tiles per core
ROWS = PER_CORE // F                    # dram view rows

_CACHE = {}


def _build_program():
    import concourse.bacc as bacc
    import concourse.tile as tile
    from concourse import mybir

    f32 = mybir.dt.float32
    AF = mybir.ActivationFunctionType
    ALU = mybir.AluOpType

    nc = bacc.Bacc("TRN2", target_bir_lowering=False, debug=False,
                   enable_asserts=True, num_devices=N_CORES)

    inp = nc.dram_tensor("inp", [ROWS, F], f32, kind="ExternalInput").ap()
    tgt = nc.dram_tensor("tgt", [ROWS, F], f32, kind="ExternalInput").ap()
    out = nc.dram_tensor("out", [1, 8], f32, kind="ExternalOutput").ap()

    inp_t = inp.rearrange("(n p) f -> n p f", p=P)
    tgt_t = tgt.rearrange("(n p) f -> n p f", p=P)

    # Chunk plan: full-width tiles, but the last two split into 1024-wide
    # chunks so the end-of-kernel drain chain is short.
    chunks = []
    for i in range(NT - 2):
        chunks.append((i, 0, F))
    for i in (NT - 2, NT - 1):
        for c0 in range(0, F, 1024):
            chunks.append((i, c0, c0 + 1024))
    NCH = len(chunks)

    with tile.TileContext(nc) as tc:
        with tc.tile_pool(name="loads", bufs=8) as lpool, \
             tc.tile_pool(name="work", bufs=4) as wpool, \
             tc.tile_pool(name="acc", bufs=1) as apool, \
             tc.tile_pool(name="psum", bufs=1, space="PSUM") as ppool:
            bf16 = mybir.dt.bfloat16
            ones = apool.tile([P, 1], bf16)
            nc.vector.memset(ones[:], 1.0)
            ones_f = apool.tile([P, 1], f32)
            nc.vector.memset(ones_f[:], 1.0)
            accL = apool.tile([P, NCH], f32)  # per-chunk sum of ln(q)
            psum_t = ppool.tile([1, 512], f32)  # running column sums of t
            psum_m = ppool.tile([1, 512], f32)  # running column sums of t*ln(q)

            for ci, (i, c0, c1) in enumerate(chunks):
                w = c1 - c0
                p = lpool.tile([P, w], f32, tag="p")
                nc.sync.dma_start(out=p[:], in_=inp_t[i][:, c0:c1])
                t = lpool.tile([P, w], bf16, tag="t")
                nc.gpsimd.dma_start(out=t[:], in_=tgt_t[i][:, c0:c1])

                u = wpool.tile([P, w], f32, tag="u")
                nc.vector.tensor_add(u[:], p[:], t[:])
                # |1 - u| = |p + t - 1| -> p where t==1, 1-p where t==0
                nc.scalar.activation(u[:], u[:], AF.Abs, bias=1.0, scale=-1.0)
                l = wpool.tile([P, w], bf16, tag="l")
                nc.scalar.activation(l[:], u[:], AF.Ln,
                                     accum_out=accL[:, ci:ci + 1])
                m = wpool.tile([P, w], bf16, tag="m")
                nc.vector.tensor_mul(m[:], t[:], l[:])
                first, last = (ci == 0), (ci == NCH - 1)
                nj = w // 512
                for j in range(nj):
                    sl = slice(j * 512, (j + 1) * 512)
                    nc.tensor.matmul(
                        psum_t[:], ones[:], t[:, sl],
                        start=(first and j == 0),
                        stop=(last and j == nj - 1))
                    nc.tensor.matmul(
                        psum_m[:], ones[:], m[:, sl],
                        start=(first and j == 0),
                        stop=(last and j == nj - 1))

            # Epilogue: fold the per-tile partials down to 3 scalars.
            red = apool.tile([P, 1], f32)
            nc.vector.tensor_reduce(red[:, 0:1], accL[:],
                                    axis=mybir.AxisListType.X, op=ALU.add)
            psum_f = ppool.tile([1, 1], f32)
            nc.tensor.matmul(psum_f[:], ones_f[:], red[:],
                             start=True, stop=True)
            res = apool.tile([1, 8], f32)
            nc.vector.memset(res[:], 0.0)
            nc.vector.tensor_copy(res[0:1, 0:1], psum_f[0:1, :])
            nc.vector.tensor_reduce(res[0:1, 1:2], psum_m[0:1, :],
                                    axis=mybir.AxisListType.X, op=ALU.add)
            nc.vector.tensor_reduce(res[0:1, 2:3], psum_t[0:1, :],
                                    axis=mybir.AxisListType.X, op=ALU.add)
            nc.sync.dma_start(out=out[0:1, :], in_=res[:])

    nc.compile()
    return nc


def _get_program():
    if "nc" not in _CACHE:
        _CACHE["nc"] = _build_program()
    return _CACHE["nc"]


def run_on_device(input, target, trace=False, **kw):
    """Shard, run on 8 cores, return (partials [8,3], BassKernelResults)."""
    from concourse import bass_utils

    nc = _get_program()
    inp = np.ascontiguousarray(input, dtype=np.float32).reshape(N_CORES, ROWS, F)
    tgt = np.ascontiguousarray(target, dtype=np.float32).reshape(N_CORES, ROWS, F)
    in_maps = [{"inp": inp[k], "tgt": tgt[k]} for k in range(N_CORES)]
    res = bass_utils.run_bass_kernel_spmd(
        nc, in_maps, core_ids=list(range(N_CORES)), trace=trace, **kw)
    partials = np.stack([res.results[k]["out"][0, :3] for k in range(N_CORES)])
    return partials, res


def _combine(partials):
    S1 = float(np.sum(partials[:, 0].astype(np.float64)))   # sum ln(q)
    S2 = float(np.sum(partials[:, 1].astype(np.float64)))   # sum t*ln(q)
    S3 = float(np.sum(partials[:, 2].astype(np.float64)))   # sum t
    A = S2
    B = S1 - S2
    pos = S3
    neg = S_TOTAL - pos
    loss = -(neg * A + pos * B) / (float(S_TOTAL) ** 2)
    return np.asarray(loss, dtype=np.float32)


def kernel(input, target):
    partials, _ = run_on_device(input, target)
    return _combine(partials)
